# revision 27
# baseline (speedup 1.0000x reference)
"""GroupedQueryAttention Trainium2 kernel (8 NeuronCores).

Problem: B=4, S=N=2048, d_model=2048, G=16 heads, d_head=128,
RoPE (rotary_dim=512) applied to query only, key-position mask,
out = (softmax(mask(QK^T/sqrt(dh))) @ V) @ Wo^T.

Sharding: mesh = 4 batches x 2 head-halves. core_id = b*2 + h.
Each core: projections for its batch/head-half, attention for its 8 heads,
chunked (per-512-query) AllGather of context^T in fp16 overlapped with
attention on the next chunk, O-projection per chunk interleaved on PE.

Projections run in fp32r (TF32) matmuls; attention (Q/K/V post-projection,
exp weights, context) and the O-projection run in fp16 — all PSUM
accumulation stays fp32.

Projection phases use 4-psum groups double-buffered across PSUM's 8 banks
so the PSUM->SBUF copies overlap the next group's matmuls; the x tiles are
loaded twice (once per head-group) on two different DMA queues.

Softmax denominator: e-tiles are accumulated (fp16, DVE 2x) into esum,
then ONE matmul with an all-ones [128,128] lhsT produces the denominator
already broadcast across all 128 partitions; a fast approximate
reciprocal and one fused multiply normalize the context.

The final chunk's O-projection is split into local-half (starts before the
last AllGather completes) and remote-half accumulation to hide the
collective's latency at the tail.
"""
import sys
import numpy as np

sys.path.insert(0, "/opt/trn_rl_repo")

from contextlib import ExitStack

import concourse.bass as bass
import concourse.tile as tile
from concourse import bacc, mybir
from concourse.bass_utils import run_bass_kernel_spmd

FP32 = mybir.dt.float32
FP32R = mybir.dt.float32r
FP16 = mybir.dt.float16

B = 4
S = 2048          # queries per batch
N = 2048          # keys per batch
D = 2048          # d_model
G = 16            # heads
DH = 128          # head dim
RD = 512          # rotary dim
TP = 2            # head-half split
CL = D // TP      # local channels (1024)
GL = G // TP      # local heads (8)
OC = D // TP      # output cols per core (1024)
SCALE = 1.0 / float(np.sqrt(DH))
MASK_BIAS = -30000.0

KT = D // 128     # contraction k-tiles (16)
SC = S // 512     # query chunks (4)
NT = N // 128     # key tiles (16)
CT = D // 128     # context c-tiles (16)
NPRE = 2          # weight k-tiles preloaded across phase boundaries


def _build_program():
    nc = bacc.Bacc("TRN2", target_bir_lowering=False, debug=False, num_devices=8)

    # ---- external I/O (per-core contents differ; same shapes) ----
    xq = nc.dram_tensor("xq", [D, S], FP32, kind="ExternalInput").ap()    # query^T
    xk = nc.dram_tensor("xk", [D, N], FP32, kind="ExternalInput").ap()    # key^T
    xv = nc.dram_tensor("xv", [D, N], FP32, kind="ExternalInput").ap()    # value^T
    wq = nc.dram_tensor("wq", [D, CL], FP32, kind="ExternalInput").ap()   # Wq[hs,:]^T
    wk = nc.dram_tensor("wk", [D, CL], FP32, kind="ExternalInput").ap()
    wv = nc.dram_tensor("wv", [D, CL], FP32, kind="ExternalInput").ap()
    wo = nc.dram_tensor("wo", [D, OC], FP16, kind="ExternalInput").ap()   # Wo^T[:, ocs] fp16
    cosT = nc.dram_tensor("cosT", [RD // 2, S], FP16, kind="ExternalInput").ap()
    sinT = nc.dram_tensor("sinT", [RD // 2, S], FP16, kind="ExternalInput").ap()
    biasm = nc.dram_tensor("biasm", [128, NT], FP32, kind="ExternalInput").ap()
    ones_in = nc.dram_tensor("ones_in", [128, 128], FP16, kind="ExternalInput").ap()
    out = nc.dram_tensor("out", [S, OC], FP32, kind="ExternalOutput").ap()

    # tiny dummy collective fired at kernel start to absorb the first
    # collective's CC-core/ring initialization cost off the critical path
    warm_in = nc.dram_tensor("warm_in", [16, 16], FP16).ap()
    warm_out = nc.dram_tensor("warm_out", [32, 16], FP16).ap()

    # ---- DRAM scratch: per-query-chunk context halves + gathered ----
    # every chunk gathers in two halves (heads 0-3 / 4-7): the A-half fires
    # mid-chunk, halving collective payloads and keeping the pair in sync
    ct_d = [nc.dram_tensor(f"ct{i}", [CL, 512], FP16).ap() for i in range(SC)]
    ctg_h = [
        [nc.dram_tensor(f"ctg{i}{a}", [CL, 512], FP16).ap() for a in range(2)]
        for i in range(SC)
    ]

    xq_r = xq.rearrange("(kt p) s -> kt p s", p=128).bitcast(FP32R)
    xk_r = xk.rearrange("(kt p) s -> kt p s", p=128).bitcast(FP32R)
    xv_r = xv.rearrange("(kt p) s -> kt p s", p=128).bitcast(FP32R)
    wq_r = wq.rearrange("(kt p) c -> p kt c", p=128).bitcast(FP32R)
    wk_r = wk.rearrange("(kt p) c -> p kt c", p=128).bitcast(FP32R)
    wv_r = wv.rearrange("(kt p) c -> p kt c", p=128).bitcast(FP32R)
    wo_r = wo.rearrange("(ct p) c -> p ct c", p=128)
    cos_r = cosT.rearrange("(gt p) s -> p gt s", p=128)
    sin_r = sinT.rearrange("(gt p) s -> p gt s", p=128)

    with tile.TileContext(nc) as tc:
        with ExitStack() as top:
            consts = top.enter_context(tc.tile_pool(name="consts", bufs=1))
            bias_t = consts.tile([128, NT], FP32)
            ones_sq = consts.tile([128, 128], FP16)

            # persistent activation tiles (fp16): Q (post-rope), K^T, V
            qkpool = top.enter_context(tc.tile_pool(name="qkpool", bufs=1))
            q_sb = qkpool.tile([128, GL, SC, 512], FP16)   # [dh, g, sc, s]
            k_sb = qkpool.tile([128, GL, NT, 128], FP16)   # [dh, g, nt, n]
            v_sb = qkpool.tile([128, NT, CL], FP16)        # [n, nt, c]

            # wk k-tile preload: loaded during Q, consumed in phase K
            prepool = top.enter_context(tc.tile_pool(name="prepool", bufs=1))
            wk_pre = prepool.tile([128, NPRE, CL], FP32R, name="wk_pre", tag="wk_pre")

            # ---------- Phase Q: Q-projection + RoPE -> q_sb ----------
            with ExitStack() as ph:
                wpool = ph.enter_context(tc.tile_pool(name="wqpool", bufs=1))
                rpool = ph.enter_context(tc.tile_pool(name="ropepool", bufs=1))
                xpool = ph.enter_context(tc.tile_pool(name="xqpool", bufs=4))
                tpool = ph.enter_context(tc.tile_pool(name="qtmppool", bufs=2))
                rsc = ph.enter_context(tc.tile_pool(name="ropescratch", bufs=2))
                pps = ph.enter_context(tc.tile_pool(name="qps", bufs=1, space="PSUM"))

                wq_t = wpool.tile([128, KT, CL], FP32R)
                cos_t = rpool.tile([128, 2, S], FP16)
                sin_t = rpool.tile([128, 2, S], FP16)
                # weights round-robin across all 3 DGE queues, in kt order
                # (the first sc group consumes them at ~2us/tile; one queue
                # sustains only ~5us/tile)
                nc.gpsimd.collective_compute(
                    "AllGather",
                    mybir.AluOpType.bypass,
                    replica_groups=[[0, 1], [2, 3], [4, 5], [6, 7]],
                    ins=[warm_in],
                    outs=[warm_out],
                )
                qs = [nc.sync, nc.scalar, nc.gpsimd]
                for kt in range(KT):
                    qs[kt % 3].dma_start(out=wq_t[:, kt, :], in_=wq_r[:, kt, :])
                nc.scalar.dma_start(out=cos_t, in_=cos_r)
                nc.gpsimd.dma_start(out=sin_t, in_=sin_r)
                nc.sync.dma_start(out=bias_t, in_=biasm)
                nc.sync.dma_start(out=ones_sq, in_=ones_in)
                # preload first wk tiles during Q
                for kt in range(NPRE):
                    nc.gpsimd.dma_start(out=wk_pre[:, kt, :], in_=wk_r[:, kt, :])

                for sc in range(SC):
                    # two 4-psum groups; B lags A by one x tile so A's copies
                    # drain while B's tail matmuls run (no PSUM WAR stall)
                    psA = [
                        pps.tile([128, 512], FP32, name=f"qpa{j}", tag=f"qpa{j}")
                        for j in range(4)
                    ]
                    psB = [
                        pps.tile([128, 512], FP32, name=f"qpb{j}", tag=f"qpb{j}")
                        for j in range(4)
                    ]
                    x_prev = None
                    for kt in range(KT + 1):
                        if kt < KT:
                            x_t = xpool.tile([128, 512], FP32R, name="xq_t", tag="x")
                            qs[(kt + 1) % 3].dma_start(
                                out=x_t, in_=xq_r[kt][:, sc * 512:(sc + 1) * 512]
                            )
                            for j in range(4):
                                nc.tensor.matmul(
                                    out=psA[j],
                                    lhsT=wq_t[:, kt, j * 128:(j + 1) * 128],
                                    rhs=x_t,
                                    start=(kt == 0),
                                    stop=(kt == KT - 1),
                                )
                        if kt > 0:
                            for j in range(4):
                                g = 4 + j
                                nc.tensor.matmul(
                                    out=psB[j],
                                    lhsT=wq_t[:, kt - 1, g * 128:(g + 1) * 128],
                                    rhs=x_prev,
                                    start=(kt == 1),
                                    stop=(kt == KT),
                                )
                        x_prev = x_t
                    # rope heads 0-3: copy to fp16 tmp, rotate on DVE
                    tmp = tpool.tile([128, 4, 512], FP16, name="qtmp", tag="qtmp")
                    for j in range(4):
                        nc.scalar.copy(out=tmp[:, j, :], in_=psA[j])
                    ssl = slice(sc * 512, (sc + 1) * 512)
                    for j in range(4):
                        sA = rsc.tile([128, 512], FP16, name="ropeA", tag="ropeA")
                        sB = rsc.tile([128, 512], FP16, name="ropeB", tag="ropeB")
                        nc.vector.tensor_mul(
                            out=sA, in0=tmp[:, j, :], in1=cos_t[:, j % 2, ssl]
                        )
                        nc.vector.tensor_mul(
                            out=sB, in0=tmp[:, j ^ 2, :], in1=sin_t[:, j % 2, ssl]
                        )
                        if j < 2:
                            nc.vector.tensor_sub(out=q_sb[:, j, sc, :], in0=sA, in1=sB)
                        else:
                            nc.vector.tensor_add(out=q_sb[:, j, sc, :], in0=sA, in1=sB)
                    for j in range(4):
                        if j % 2 == 0:
                            nc.scalar.copy(out=q_sb[:, 4 + j, sc, :], in_=psB[j])
                        else:
                            nc.vector.tensor_copy(out=q_sb[:, 4 + j, sc, :], in_=psB[j])

            # ---------- Phases K+V (wv preload spans both) ----------
            kv = top.enter_context(ExitStack())
            wvprepool = kv.enter_context(tc.tile_pool(name="wvprepool", bufs=1))
            wv_pre = wvprepool.tile([128, NPRE, CL], FP32R, name="wv_pre", tag="wv_pre")

            # ---------- Phase K: K-projection -> k_sb ----------
            with ExitStack() as ph:
                wpool = ph.enter_context(tc.tile_pool(name="wkpool", bufs=1))
                xpool = ph.enter_context(tc.tile_pool(name="xkpool", bufs=4))
                pps = ph.enter_context(tc.tile_pool(name="kps", bufs=1, space="PSUM"))

                wk_t = wpool.tile([128, KT, CL], FP32R)
                qs = [nc.sync, nc.scalar, nc.gpsimd]
                for kt in range(NPRE, KT):
                    qs[kt % 3].dma_start(out=wk_t[:, kt, :], in_=wk_r[:, kt, :])
                # preload first wv tiles during K
                for kt in range(NPRE):
                    nc.gpsimd.dma_start(out=wv_pre[:, kt, :], in_=wv_r[:, kt, :])

                def wk_sl(kt):
                    return wk_pre[:, kt, :] if kt < NPRE else wk_t[:, kt, :]

                for nch in range(N // 512):
                    psA = [
                        pps.tile([128, 512], FP32, name=f"kpa{j}", tag=f"kpa{j}")
                        for j in range(4)
                    ]
                    psB = [
                        pps.tile([128, 512], FP32, name=f"kpb{j}", tag=f"kpb{j}")
                        for j in range(4)
                    ]
                    x_prev = None
                    for kt in range(KT + 1):
                        if kt < KT:
                            x_t = xpool.tile([128, 512], FP32R, name="xk_t", tag="x")
                            qs[(kt + 1) % 3].dma_start(
                                out=x_t, in_=xk_r[kt][:, nch * 512:(nch + 1) * 512]
                            )
                            for j in range(4):
                                nc.tensor.matmul(
                                    out=psA[j],
                                    lhsT=wk_sl(kt)[:, j * 128:(j + 1) * 128],
                                    rhs=x_t,
                                    start=(kt == 0),
                                    stop=(kt == KT - 1),
                                )
                        if kt > 0:
                            for j in range(4):
                                g = 4 + j
                                nc.tensor.matmul(
                                    out=psB[j],
                                    lhsT=wk_sl(kt - 1)[:, g * 128:(g + 1) * 128],
                                    rhs=x_prev,
                                    start=(kt == 1),
                                    stop=(kt == KT),
                                )
                        x_prev = x_t
                    for half, ps in ((0, psA), (1, psB)):
                        for j in range(4):
                            g = half * 4 + j
                            src = ps[j].rearrange("p (a b) -> p a b", b=128)
                            dst = k_sb[:, g, nch * 4:(nch + 1) * 4, :]
                            if j % 2 == 0:
                                nc.scalar.copy(out=dst, in_=src)
                            else:
                                nc.vector.tensor_copy(out=dst, in_=src)

            # ---------- Phase V: V-projection -> v_sb (natural layout) ----------
            with ExitStack() as ph:
                wpool = ph.enter_context(tc.tile_pool(name="wvpool", bufs=1))
                xpool = ph.enter_context(tc.tile_pool(name="xvpool", bufs=4))
                pps = ph.enter_context(tc.tile_pool(name="vps", bufs=1, space="PSUM"))

                wv_t = wpool.tile([128, KT, CL], FP32R)
                qs = [nc.sync, nc.scalar, nc.gpsimd]
                for kt in range(NPRE, KT):
                    qs[kt % 3].dma_start(out=wv_t[:, kt, :], in_=wv_r[:, kt, :])

                def wv_sl(kt):
                    return wv_pre[:, kt, :] if kt < NPRE else wv_t[:, kt, :]

                for nt4 in range(N // 512):
                    # psums (j2, cc): A covers n-tile pair 0-1, B (lagged) 2-3
                    psA = [
                        pps.tile([128, 512], FP32, name=f"vpa{j}", tag=f"vpa{j}")
                        for j in range(4)
                    ]
                    psB = [
                        pps.tile([128, 512], FP32, name=f"vpb{j}", tag=f"vpb{j}")
                        for j in range(4)
                    ]
                    x_prev = None
                    for kt in range(KT + 1):
                        if kt < KT:
                            x_t = xpool.tile([128, 512], FP32R, name="xv_t", tag="x")
                            qs[(kt + 1) % 3].dma_start(
                                out=x_t, in_=xv_r[kt][:, nt4 * 512:(nt4 + 1) * 512]
                            )
                            for j in range(4):
                                j2, cc = j // 2, j % 2
                                nc.tensor.matmul(
                                    out=psA[j],
                                    lhsT=x_t[:, j2 * 128:(j2 + 1) * 128],
                                    rhs=wv_sl(kt)[:, cc * 512:(cc + 1) * 512],
                                    start=(kt == 0),
                                    stop=(kt == KT - 1),
                                )
                        if kt > 0:
                            for j in range(4):
                                j2, cc = j // 2, j % 2
                                jj = 2 + j2
                                nc.tensor.matmul(
                                    out=psB[j],
                                    lhsT=x_prev[:, jj * 128:(jj + 1) * 128],
                                    rhs=wv_sl(kt - 1)[:, cc * 512:(cc + 1) * 512],
                                    start=(kt == 1),
                                    stop=(kt == KT),
                                )
                        x_prev = x_t
                    for half, ps in ((0, psA), (1, psB)):
                        for j in range(4):
                            j2, cc = j // 2, j % 2
                            nt = nt4 * 4 + half * 2 + j2
                            dst = v_sb[:, nt, cc * 512:(cc + 1) * 512]
                            if j % 2 == 0:
                                nc.scalar.copy(out=dst, in_=ps[j])
                            else:
                                nc.vector.tensor_copy(out=dst, in_=ps[j])

            # re-sync the pair just before attention so the first real
            # gather doesn't absorb QKV-phase skew between the two cores
            nc.gpsimd.collective_compute(
                "AllGather",
                mybir.AluOpType.bypass,
                replica_groups=[[0, 1], [2, 3], [4, 5], [6, 7]],
                ins=[warm_in],
                outs=[warm_out],
            )

            kv.close()

            # ---------- Attention (per query chunk) + chunked gather + O ----------
            with ExitStack() as ph:
                wpool = ph.enter_context(tc.tile_pool(name="wopool", bufs=1))
                epool = ph.enter_context(tc.tile_pool(name="epool", bufs=4))
                espool = ph.enter_context(tc.tile_pool(name="espool", bufs=2))
                rpool = ph.enter_context(tc.tile_pool(name="rpool", bufs=2))
                cpool = ph.enter_context(tc.tile_pool(name="cpool", bufs=2))
                csbpool = ph.enter_context(tc.tile_pool(name="csbpool", bufs=2))
                osbpool = ph.enter_context(tc.tile_pool(name="osbpool", bufs=2))
                sps = ph.enter_context(tc.tile_pool(name="sps", bufs=2, space="PSUM"))
                ups = ph.enter_context(tc.tile_pool(name="ups", bufs=2, space="PSUM"))
                dps = ph.enter_context(tc.tile_pool(name="dps", bufs=2, space="PSUM"))
                ops = ph.enter_context(tc.tile_pool(name="ops", bufs=1, space="PSUM"))

                wo_t = wpool.tile([128, CT, OC], FP16)
                for ct in range(CT):
                    nc.gpsimd.dma_start(out=wo_t[:, ct, :], in_=wo_r[:, ct, :])

                def gather(in_ap, out_ap):
                    nc.gpsimd.collective_compute(
                        "AllGather",
                        mybir.AluOpType.bypass,
                        replica_groups=[[0, 1], [2, 3], [4, 5], [6, 7]],
                        ins=[in_ap],
                        outs=[out_ap],
                    )

                def emit_attn(sc):
                    for g in range(GL):
                        u_ps = ups.tile([128, 512], FP32, name="u_ps", tag="u")
                        esum = espool.tile([128, 512], FP16, name="esum", tag="es")
                        for nt in range(NT):
                            s_ps = sps.tile([128, 512], FP32, name="s_ps", tag="s")
                            nc.tensor.matmul(
                                out=s_ps,
                                lhsT=k_sb[:, g, nt, :],
                                rhs=q_sb[:, g, sc, :],
                                start=True,
                                stop=True,
                            )
                            e_t = epool.tile([128, 512], FP16, name="e_t", tag="e")
                            nc.scalar.activation(
                                out=e_t, in_=s_ps,
                                func=mybir.ActivationFunctionType.Exp,
                                bias=bias_t[:, nt:nt + 1], scale=SCALE,
                            )
                            with nc.allow_low_precision(reason="fp16 esum accumulate"):
                                if nt == 0:
                                    nc.vector.tensor_copy(out=esum, in_=e_t)
                                else:
                                    nc.vector.tensor_add(out=esum, in0=esum, in1=e_t)
                            nc.tensor.matmul(
                                out=u_ps,
                                lhsT=v_sb[:, nt, g * 128:(g + 1) * 128],
                                rhs=e_t,
                                start=(nt == 0),
                                stop=(nt == NT - 1),
                            )
                        # denominator, broadcast across partitions in one matmul
                        d_ps = dps.tile([128, 512], FP32, name="d_ps", tag="d")
                        nc.tensor.matmul(
                            out=d_ps, lhsT=ones_sq, rhs=esum, start=True, stop=True
                        )
                        r_t = rpool.tile([128, 512], FP32, name="r_t", tag="r")
                        nc.vector.reciprocal_approx_fast(out=r_t, in_=d_ps)
                        c_t = cpool.tile([128, 512], FP16, name="c_t", tag="c")
                        nc.vector.tensor_mul(out=c_t, in0=u_ps, in1=r_t)
                        nc.sync.dma_start(
                            out=ct_d[sc][g * 128:(g + 1) * 128, :], in_=c_t
                        )
                        if g == 3:
                            gather(ct_d[sc][0:512, :], ctg_h[sc][0])
                    gather(ct_d[sc][512:1024, :], ctg_h[sc][1])

                def emit_o(j):
                    """O-projection for query chunk j, gathered as two
                    half-tensors (heads 0-3 / 4-7 of both ranks): global
                    c-tile t lives in half (t % 8) >= 4 at tile
                    (t // 8) * 4 + (t % 4)."""
                    ab_r = [
                        t.rearrange("(ct p) s -> p ct s", p=128) for t in ctg_h[j]
                    ]
                    for st4 in range(4):
                        st = j * 4 + st4
                        ssl = slice(st4 * 128, (st4 + 1) * 128)
                        psums = []
                        for cc in range(2):
                            p = ops.tile([128, 512], FP32, name=f"op{cc}", tag=f"op{cc}")
                            psums.append(p)
                        cA = csbpool.tile(
                            [128, CT // 2, 128], FP16, name="cA", tag="c_sb"
                        )
                        nc.sync.dma_start(out=cA, in_=ab_r[0][:, :, ssl])
                        cB = csbpool.tile(
                            [128, CT // 2, 128], FP16, name="cB", tag="c_sb2"
                        )
                        nc.sync.dma_start(out=cB, in_=ab_r[1][:, :, ssl])

                        def csl(t):
                            half = cA if (t % 8) < 4 else cB
                            return half[:, (t // 8) * 4 + (t % 4), :]

                        # A-half tiles first so PE starts before gather B lands
                        order = [0, 1, 2, 3, 8, 9, 10, 11, 4, 5, 6, 7, 12, 13, 14, 15]
                        for i, ct in enumerate(order):
                            for cc in range(2):
                                nc.tensor.matmul(
                                    out=psums[cc],
                                    lhsT=csl(ct),
                                    rhs=wo_t[:, ct, cc * 512:(cc + 1) * 512],
                                    start=(i == 0),
                                    stop=(i == CT - 1),
                                )
                        o_sb = osbpool.tile([128, OC], FP32, name="o_sb", tag="o_sb")
                        nc.scalar.copy(out=o_sb[:, 0:512], in_=psums[0])
                        nc.vector.tensor_copy(out=o_sb[:, 512:1024], in_=psums[1])
                        nc.sync.dma_start(out=out[st * 128:(st + 1) * 128, :], in_=o_sb)

                emit_attn(0)
                emit_attn(1)
                emit_o(0)
                emit_attn(2)
                emit_o(1)
                emit_attn(3)
                emit_o(2)
                emit_o(3)

    nc.compile()
    return nc


_NC_CACHE = {}


def _get_program():
    if "nc" not in _NC_CACHE:
        _NC_CACHE["nc"] = _build_program()
    return _NC_CACHE["nc"]


def kernel(query, key, value, mask, position_ids, Wq, Wk, Wv, Wo, **kw):
    query = np.asarray(query, dtype=np.float32)
    key = np.asarray(key, dtype=np.float32)
    value = np.asarray(value, dtype=np.float32)
    mask = np.asarray(mask)
    position_ids = np.asarray(position_ids)
    Wq = np.asarray(Wq, dtype=np.float32)
    Wk = np.asarray(Wk, dtype=np.float32)
    Wv = np.asarray(Wv, dtype=np.float32)
    Wo = np.asarray(Wo, dtype=np.float32)

    # rope tables from actual position_ids (applied to query only)
    pos = position_ids.astype(np.float64)  # (S,)
    freq = np.arange(0, RD, 2, dtype=np.float64)
    inv_freq = 1.0 / (10000.0 ** (freq / RD))  # (RD/2,)
    pe = pos[:, None] * inv_freq[None, :]      # (S, RD/2=256)
    cosT_half = np.ascontiguousarray(np.cos(pe).T.astype(np.float16))  # (256, S)
    sinT_half = np.ascontiguousarray(np.sin(pe).T.astype(np.float16))
    cosT_id = np.ones((RD // 2, S), np.float16)
    sinT_id = np.zeros((RD // 2, S), np.float16)

    ones_sq = np.ones((128, 128), np.float16)
    WoT = Wo.T.astype(np.float16)

    in_maps = []
    for core in range(8):
        b, h = core // 2, core % 2
        hs = slice(h * CL, (h + 1) * CL)
        biasm = np.where(mask[b] == 0, np.float32(MASK_BIAS), np.float32(0.0))
        in_maps.append({
            "xq": np.ascontiguousarray(query[b].T),
            "xk": np.ascontiguousarray(key[b].T),
            "xv": np.ascontiguousarray(value[b].T),
            "wq": np.ascontiguousarray(Wq[hs, :].T),
            "wk": np.ascontiguousarray(Wk[hs, :].T),
            "wv": np.ascontiguousarray(Wv[hs, :].T),
            "wo": np.ascontiguousarray(WoT[:, hs]),
            "cosT": cosT_half if h == 0 else cosT_id,
            "sinT": sinT_half if h == 0 else sinT_id,
            "biasm": np.ascontiguousarray(biasm.reshape(NT, 128).T),
            "ones_in": ones_sq,
        })

    nc = _get_program()
    res = run_bass_kernel_spmd(nc, in_maps, core_ids=list(range(8)))
    _NC_CACHE["last_res"] = res

    out = np.empty((B, S, D), np.float32)
    for core in range(8):
        b, h = core // 2, core % 2
        out[b][:, h * OC:(h + 1) * OC] = res.results[core]["out"]
    return out


# revision 30
# speedup vs baseline: 1.0773x; 1.0773x over previous
"""GroupedQueryAttention Trainium2 kernel (8 NeuronCores).

Problem: B=4, S=N=2048, d_model=2048, G=16 heads, d_head=128,
RoPE (rotary_dim=512) applied to query only, key-position mask,
out = (softmax(mask(QK^T/sqrt(dh))) @ V) @ Wo^T.

Sharding: mesh = 4 batches x 2 head-halves. core_id = b*2 + h.
Each core: projections for its batch/head-half, attention for its 8 heads,
chunked (per-512-query) AllGather of context^T in fp16 overlapped with
attention on the next chunk, O-projection per chunk interleaved on PE.

Projections run in fp32r (TF32) matmuls; attention (Q/K/V post-projection,
exp weights, context) and the O-projection run in fp16 — all PSUM
accumulation stays fp32.

Projection phases use 4-psum groups double-buffered across PSUM's 8 banks
so the PSUM->SBUF copies overlap the next group's matmuls; the x tiles are
loaded twice (once per head-group) on two different DMA queues.

Softmax denominator: e-tiles are accumulated (fp16, DVE 2x) into esum,
then ONE matmul with an all-ones [128,128] lhsT produces the denominator
already broadcast across all 128 partitions; a fast approximate
reciprocal and one fused multiply normalize the context.

The final chunk's O-projection is split into local-half (starts before the
last AllGather completes) and remote-half accumulation to hide the
collective's latency at the tail.
"""
import sys
import numpy as np

sys.path.insert(0, "/opt/trn_rl_repo")

from contextlib import ExitStack

import concourse.bass as bass
import concourse.tile as tile
from concourse import bacc, mybir
from concourse.bass_utils import run_bass_kernel_spmd

FP32 = mybir.dt.float32
FP32R = mybir.dt.float32r
FP16 = mybir.dt.float16

B = 4
S = 2048          # queries per batch
N = 2048          # keys per batch
D = 2048          # d_model
G = 16            # heads
DH = 128          # head dim
RD = 512          # rotary dim
TP = 2            # head-half split
CL = D // TP      # local channels (1024)
GL = G // TP      # local heads (8)
OC = D // TP      # output cols per core (1024)
SCALE = 1.0 / float(np.sqrt(DH))
MASK_BIAS = -30000.0

KT = D // 128     # contraction k-tiles (16)
SC = S // 512     # query chunks (4)
NT = N // 128     # key tiles (16)
CT = D // 128     # context c-tiles (16)
NPRE = 3          # weight k-tiles preloaded across phase boundaries


def _build_program():
    nc = bacc.Bacc("TRN2", target_bir_lowering=False, debug=False, num_devices=8)

    # ---- external I/O (per-core contents differ; same shapes) ----
    xq = nc.dram_tensor("xq", [D, S], FP32, kind="ExternalInput").ap()    # query^T
    xk = nc.dram_tensor("xk", [D, N], FP32, kind="ExternalInput").ap()    # key^T
    xv = nc.dram_tensor("xv", [D, N], FP32, kind="ExternalInput").ap()    # value^T
    wq = nc.dram_tensor("wq", [D, CL], FP32, kind="ExternalInput").ap()   # Wq[hs,:]^T
    wk = nc.dram_tensor("wk", [D, CL], FP32, kind="ExternalInput").ap()
    wv = nc.dram_tensor("wv", [D, CL], FP32, kind="ExternalInput").ap()
    wo = nc.dram_tensor("wo", [D, OC], FP16, kind="ExternalInput").ap()   # Wo^T[:, ocs] fp16
    cosT = nc.dram_tensor("cosT", [RD // 2, S], FP16, kind="ExternalInput").ap()
    sinT = nc.dram_tensor("sinT", [RD // 2, S], FP16, kind="ExternalInput").ap()
    biasm = nc.dram_tensor("biasm", [128, NT], FP32, kind="ExternalInput").ap()
    ones_in = nc.dram_tensor("ones_in", [128, 128], FP16, kind="ExternalInput").ap()
    out = nc.dram_tensor("out", [S, OC], FP32, kind="ExternalOutput").ap()

    # tiny dummy collective fired at kernel start to absorb the first
    # collective's CC-core/ring initialization cost off the critical path
    warm_in = nc.dram_tensor("warm_in", [16, 16], FP16).ap()
    warm_out = nc.dram_tensor("warm_out", [32, 16], FP16).ap()

    # ---- DRAM scratch: per-query-chunk context halves + gathered ----
    # chunks 0-2 gather in two halves (heads 0-3 / 4-7) fired mid-chunk;
    # the last chunk gathers in four quarters so the final piece is small
    # and the tail O-projection starts sooner
    NPIECE = [2, 2, 2, 4]
    ct_d = [nc.dram_tensor(f"ct{i}", [CL, 512], FP16).ap() for i in range(SC)]
    ctg_h = [
        [
            nc.dram_tensor(
                f"ctg{i}_{a}", [2 * (CL // NPIECE[i]), 512], FP16
            ).ap()
            for a in range(NPIECE[i])
        ]
        for i in range(SC)
    ]

    xq_r = xq.rearrange("(kt p) s -> kt p s", p=128).bitcast(FP32R)
    xk_r = xk.rearrange("(kt p) s -> kt p s", p=128).bitcast(FP32R)
    xv_r = xv.rearrange("(kt p) s -> kt p s", p=128).bitcast(FP32R)
    wq_r = wq.rearrange("(kt p) c -> p kt c", p=128).bitcast(FP32R)
    wk_r = wk.rearrange("(kt p) c -> p kt c", p=128).bitcast(FP32R)
    wv_r = wv.rearrange("(kt p) c -> p kt c", p=128).bitcast(FP32R)
    wo_r = wo.rearrange("(ct p) c -> p ct c", p=128)
    cos_r = cosT.rearrange("(gt p) s -> p gt s", p=128)
    sin_r = sinT.rearrange("(gt p) s -> p gt s", p=128)

    with tile.TileContext(nc) as tc:
        with ExitStack() as top:
            consts = top.enter_context(tc.tile_pool(name="consts", bufs=1))
            bias_t = consts.tile([128, NT], FP32)
            ones_sq = consts.tile([128, 128], FP16)

            # persistent activation tiles (fp16): Q (post-rope), K^T, V
            qkpool = top.enter_context(tc.tile_pool(name="qkpool", bufs=1))
            q_sb = qkpool.tile([128, GL, SC, 512], FP16)   # [dh, g, sc, s]
            k_sb = qkpool.tile([128, GL, NT, 128], FP16)   # [dh, g, nt, n]
            v_sb = qkpool.tile([128, NT, CL], FP16)        # [n, nt, c]

            # wk k-tile preload: loaded during Q, consumed in phase K
            prepool = top.enter_context(tc.tile_pool(name="prepool", bufs=1))
            wk_pre = prepool.tile([128, NPRE, CL], FP32R, name="wk_pre", tag="wk_pre")

            # ---------- Phase Q: Q-projection + RoPE -> q_sb ----------
            with ExitStack() as ph:
                wpool = ph.enter_context(tc.tile_pool(name="wqpool", bufs=1))
                rpool = ph.enter_context(tc.tile_pool(name="ropepool", bufs=1))
                xpool = ph.enter_context(tc.tile_pool(name="xqpool", bufs=4))
                tpool = ph.enter_context(tc.tile_pool(name="qtmppool", bufs=1))
                rsc = ph.enter_context(tc.tile_pool(name="ropescratch", bufs=2))
                pps = ph.enter_context(tc.tile_pool(name="qps", bufs=1, space="PSUM"))

                wq_t = wpool.tile([128, KT, CL], FP32R)
                cos_t = rpool.tile([128, 2, S], FP16)
                sin_t = rpool.tile([128, 2, S], FP16)
                # first two wq tiles ride the (empty) x queues; the rest
                # stream on the dedicated gpsimd weight queue
                nc.sync.dma_start(out=wq_t[:, 0, :], in_=wq_r[:, 0, :])
                nc.scalar.dma_start(out=wq_t[:, 1, :], in_=wq_r[:, 1, :])
                for kt in range(2, KT):
                    nc.gpsimd.dma_start(out=wq_t[:, kt, :], in_=wq_r[:, kt, :])
                nc.sync.dma_start(out=bias_t, in_=biasm)
                nc.sync.dma_start(out=ones_sq, in_=ones_in)
                for kt in range(NPRE):
                    nc.gpsimd.dma_start(out=wk_pre[:, kt, :], in_=wk_r[:, kt, :])
                nc.gpsimd.dma_start(out=cos_t, in_=cos_r)
                nc.gpsimd.dma_start(out=sin_t, in_=sin_r)
                # warm the CC path while projections run
                nc.gpsimd.collective_compute(
                    "AllGather",
                    mybir.AluOpType.bypass,
                    replica_groups=[[0, 1], [2, 3], [4, 5], [6, 7]],
                    ins=[warm_in],
                    outs=[warm_out],
                )

                for sc in range(SC):
                    # two 4-psum groups; B lags A by one x tile so A's copies
                    # drain while B's tail matmuls run (no PSUM WAR stall)
                    psA = [
                        pps.tile([128, 512], FP32, name=f"qpa{j}", tag=f"qpa{j}")
                        for j in range(4)
                    ]
                    psB = [
                        pps.tile([128, 512], FP32, name=f"qpb{j}", tag=f"qpb{j}")
                        for j in range(4)
                    ]
                    x_prev = None
                    for kt in range(KT + 1):
                        if kt < KT:
                            x_t = xpool.tile([128, 512], FP32R, name="xq_t", tag="x")
                            (nc.sync if kt % 2 == 0 else nc.scalar).dma_start(
                                out=x_t, in_=xq_r[kt][:, sc * 512:(sc + 1) * 512]
                            )
                            for j in range(4):
                                nc.tensor.matmul(
                                    out=psA[j],
                                    lhsT=wq_t[:, kt, j * 128:(j + 1) * 128],
                                    rhs=x_t,
                                    start=(kt == 0),
                                    stop=(kt == KT - 1),
                                )
                        if kt > 0:
                            for j in range(4):
                                g = 4 + j
                                nc.tensor.matmul(
                                    out=psB[j],
                                    lhsT=wq_t[:, kt - 1, g * 128:(g + 1) * 128],
                                    rhs=x_prev,
                                    start=(kt == 1),
                                    stop=(kt == KT),
                                )
                        x_prev = x_t
                    # rope heads 0-3: copy to fp16 tmp, rotate on DVE
                    tmp = tpool.tile([128, 4, 512], FP16, name="qtmp", tag="qtmp")
                    for j in range(4):
                        nc.scalar.copy(out=tmp[:, j, :], in_=psA[j])
                    ssl = slice(sc * 512, (sc + 1) * 512)
                    for j in range(4):
                        sA = rsc.tile([128, 512], FP16, name="ropeA", tag="ropeA")
                        sB = rsc.tile([128, 512], FP16, name="ropeB", tag="ropeB")
                        nc.vector.tensor_mul(
                            out=sA, in0=tmp[:, j, :], in1=cos_t[:, j % 2, ssl]
                        )
                        nc.vector.tensor_mul(
                            out=sB, in0=tmp[:, j ^ 2, :], in1=sin_t[:, j % 2, ssl]
                        )
                        if j < 2:
                            nc.vector.tensor_sub(out=q_sb[:, j, sc, :], in0=sA, in1=sB)
                        else:
                            nc.vector.tensor_add(out=q_sb[:, j, sc, :], in0=sA, in1=sB)
                    for j in range(4):
                        if j % 2 == 0:
                            nc.scalar.copy(out=q_sb[:, 4 + j, sc, :], in_=psB[j])
                        else:
                            nc.vector.tensor_copy(out=q_sb[:, 4 + j, sc, :], in_=psB[j])

            # ---------- Phases K+V (wv preload spans both) ----------
            kv = top.enter_context(ExitStack())
            wvprepool = kv.enter_context(tc.tile_pool(name="wvprepool", bufs=1))
            wv_pre = wvprepool.tile([128, NPRE, CL], FP32R, name="wv_pre", tag="wv_pre")

            # ---------- Phase K: K-projection -> k_sb ----------
            with ExitStack() as ph:
                wpool = ph.enter_context(tc.tile_pool(name="wkpool", bufs=1))
                xpool = ph.enter_context(tc.tile_pool(name="xkpool", bufs=4))
                pps = ph.enter_context(tc.tile_pool(name="kps", bufs=1, space="PSUM"))

                wk_t = wpool.tile([128, KT, CL], FP32R)
                for kt in range(NPRE, KT):
                    nc.gpsimd.dma_start(out=wk_t[:, kt, :], in_=wk_r[:, kt, :])
                # preload first wv tiles during K
                for kt in range(NPRE):
                    nc.gpsimd.dma_start(out=wv_pre[:, kt, :], in_=wv_r[:, kt, :])

                def wk_sl(kt):
                    return wk_pre[:, kt, :] if kt < NPRE else wk_t[:, kt, :]

                for nch in range(N // 512):
                    psA = [
                        pps.tile([128, 512], FP32, name=f"kpa{j}", tag=f"kpa{j}")
                        for j in range(4)
                    ]
                    psB = [
                        pps.tile([128, 512], FP32, name=f"kpb{j}", tag=f"kpb{j}")
                        for j in range(4)
                    ]
                    x_prev = None
                    for kt in range(KT + 1):
                        if kt < KT:
                            x_t = xpool.tile([128, 512], FP32R, name="xk_t", tag="x")
                            (nc.sync if kt % 2 == 0 else nc.scalar).dma_start(
                                out=x_t, in_=xk_r[kt][:, nch * 512:(nch + 1) * 512]
                            )
                            for j in range(4):
                                nc.tensor.matmul(
                                    out=psA[j],
                                    lhsT=wk_sl(kt)[:, j * 128:(j + 1) * 128],
                                    rhs=x_t,
                                    start=(kt == 0),
                                    stop=(kt == KT - 1),
                                )
                        if kt > 0:
                            for j in range(4):
                                g = 4 + j
                                nc.tensor.matmul(
                                    out=psB[j],
                                    lhsT=wk_sl(kt - 1)[:, g * 128:(g + 1) * 128],
                                    rhs=x_prev,
                                    start=(kt == 1),
                                    stop=(kt == KT),
                                )
                        x_prev = x_t
                    for half, ps in ((0, psA), (1, psB)):
                        for j in range(4):
                            g = half * 4 + j
                            src = ps[j].rearrange("p (a b) -> p a b", b=128)
                            dst = k_sb[:, g, nch * 4:(nch + 1) * 4, :]
                            if j % 2 == 0:
                                nc.scalar.copy(out=dst, in_=src)
                            else:
                                nc.vector.tensor_copy(out=dst, in_=src)

            # ---------- Phase V: V-projection -> v_sb (natural layout) ----------
            with ExitStack() as ph:
                wpool = ph.enter_context(tc.tile_pool(name="wvpool", bufs=1))
                xpool = ph.enter_context(tc.tile_pool(name="xvpool", bufs=4))
                pps = ph.enter_context(tc.tile_pool(name="vps", bufs=1, space="PSUM"))

                wv_t = wpool.tile([128, KT, CL], FP32R)
                for kt in range(NPRE, KT):
                    nc.gpsimd.dma_start(out=wv_t[:, kt, :], in_=wv_r[:, kt, :])

                def wv_sl(kt):
                    return wv_pre[:, kt, :] if kt < NPRE else wv_t[:, kt, :]

                for nt4 in range(N // 512):
                    # psums (j2, cc): A covers n-tile pair 0-1, B (lagged) 2-3
                    psA = [
                        pps.tile([128, 512], FP32, name=f"vpa{j}", tag=f"vpa{j}")
                        for j in range(4)
                    ]
                    psB = [
                        pps.tile([128, 512], FP32, name=f"vpb{j}", tag=f"vpb{j}")
                        for j in range(4)
                    ]
                    x_prev = None
                    for kt in range(KT + 1):
                        if kt < KT:
                            x_t = xpool.tile([128, 512], FP32R, name="xv_t", tag="x")
                            (nc.sync if kt % 2 == 0 else nc.scalar).dma_start(
                                out=x_t, in_=xv_r[kt][:, nt4 * 512:(nt4 + 1) * 512]
                            )
                            for j in range(4):
                                j2, cc = j // 2, j % 2
                                nc.tensor.matmul(
                                    out=psA[j],
                                    lhsT=x_t[:, j2 * 128:(j2 + 1) * 128],
                                    rhs=wv_sl(kt)[:, cc * 512:(cc + 1) * 512],
                                    start=(kt == 0),
                                    stop=(kt == KT - 1),
                                )
                        if kt > 0:
                            for j in range(4):
                                j2, cc = j // 2, j % 2
                                jj = 2 + j2
                                nc.tensor.matmul(
                                    out=psB[j],
                                    lhsT=x_prev[:, jj * 128:(jj + 1) * 128],
                                    rhs=wv_sl(kt - 1)[:, cc * 512:(cc + 1) * 512],
                                    start=(kt == 1),
                                    stop=(kt == KT),
                                )
                        x_prev = x_t
                    for half, ps in ((0, psA), (1, psB)):
                        for j in range(4):
                            j2, cc = j // 2, j % 2
                            nt = nt4 * 4 + half * 2 + j2
                            dst = v_sb[:, nt, cc * 512:(cc + 1) * 512]
                            if j % 2 == 0:
                                nc.scalar.copy(out=dst, in_=ps[j])
                            else:
                                nc.vector.tensor_copy(out=dst, in_=ps[j])

            # re-sync the pair just before attention so the first real
            # gather doesn't absorb QKV-phase skew between the two cores
            nc.gpsimd.collective_compute(
                "AllGather",
                mybir.AluOpType.bypass,
                replica_groups=[[0, 1], [2, 3], [4, 5], [6, 7]],
                ins=[warm_in],
                outs=[warm_out],
            )

            kv.close()

            # ---------- Attention (per query chunk) + chunked gather + O ----------
            with ExitStack() as ph:
                wpool = ph.enter_context(tc.tile_pool(name="wopool", bufs=1))
                epool = ph.enter_context(tc.tile_pool(name="epool", bufs=4))
                espool = ph.enter_context(tc.tile_pool(name="espool", bufs=2))
                rpool = ph.enter_context(tc.tile_pool(name="rpool", bufs=2))
                cpool = ph.enter_context(tc.tile_pool(name="cpool", bufs=2))
                csbpool = ph.enter_context(tc.tile_pool(name="csbpool", bufs=2))
                osbpool = ph.enter_context(tc.tile_pool(name="osbpool", bufs=2))
                sps = ph.enter_context(tc.tile_pool(name="sps", bufs=2, space="PSUM"))
                ups = ph.enter_context(tc.tile_pool(name="ups", bufs=2, space="PSUM"))
                dps = ph.enter_context(tc.tile_pool(name="dps", bufs=2, space="PSUM"))
                ops = ph.enter_context(tc.tile_pool(name="ops", bufs=1, space="PSUM"))

                wo_t = wpool.tile([128, CT, OC], FP16)
                for ct in range(CT):
                    nc.gpsimd.dma_start(out=wo_t[:, ct, :], in_=wo_r[:, ct, :])

                def gather(in_ap, out_ap):
                    nc.gpsimd.collective_compute(
                        "AllGather",
                        mybir.AluOpType.bypass,
                        replica_groups=[[0, 1], [2, 3], [4, 5], [6, 7]],
                        ins=[in_ap],
                        outs=[out_ap],
                    )

                def emit_attn(sc):
                    npc = NPIECE[sc]
                    hp = GL // npc          # heads per gather piece
                    for g in range(GL):
                        u_ps = ups.tile([128, 512], FP32, name="u_ps", tag="u")
                        esum = espool.tile([128, 512], FP16, name="esum", tag="es")
                        for nt in range(NT):
                            s_ps = sps.tile([128, 512], FP32, name="s_ps", tag="s")
                            nc.tensor.matmul(
                                out=s_ps,
                                lhsT=k_sb[:, g, nt, :],
                                rhs=q_sb[:, g, sc, :],
                                start=True,
                                stop=True,
                            )
                            e_t = epool.tile([128, 512], FP16, name="e_t", tag="e")
                            nc.scalar.activation(
                                out=e_t, in_=s_ps,
                                func=mybir.ActivationFunctionType.Exp,
                                bias=bias_t[:, nt:nt + 1], scale=SCALE,
                            )
                            with nc.allow_low_precision(reason="fp16 esum accumulate"):
                                if nt == 0:
                                    nc.vector.tensor_copy(out=esum, in_=e_t)
                                else:
                                    nc.vector.tensor_add(out=esum, in0=esum, in1=e_t)
                            nc.tensor.matmul(
                                out=u_ps,
                                lhsT=v_sb[:, nt, g * 128:(g + 1) * 128],
                                rhs=e_t,
                                start=(nt == 0),
                                stop=(nt == NT - 1),
                            )
                        # denominator, broadcast across partitions in one matmul
                        d_ps = dps.tile([128, 512], FP32, name="d_ps", tag="d")
                        nc.tensor.matmul(
                            out=d_ps, lhsT=ones_sq, rhs=esum, start=True, stop=True
                        )
                        r_t = rpool.tile([128, 512], FP32, name="r_t", tag="r")
                        nc.vector.reciprocal_approx_fast(out=r_t, in_=d_ps)
                        c_t = cpool.tile([128, 512], FP16, name="c_t", tag="c")
                        nc.vector.tensor_mul(out=c_t, in0=u_ps, in1=r_t)
                        nc.sync.dma_start(
                            out=ct_d[sc][g * 128:(g + 1) * 128, :], in_=c_t
                        )
                        if g % hp == hp - 1 and g != GL - 1:
                            p = g // hp
                            gather(
                                ct_d[sc][p * hp * 128:(p + 1) * hp * 128, :],
                                ctg_h[sc][p],
                            )
                    gather(
                        ct_d[sc][(GL - hp) * 128:GL * 128, :], ctg_h[sc][npc - 1]
                    )

                def emit_o(j):
                    """O-projection for query chunk j. Piece p holds heads
                    [p*hp, (p+1)*hp) of both ranks: global c-tile t (rank
                    t//8, head t%8) lives in piece (t%8)//hp at tile
                    (t//8)*hp + t%hp."""
                    npc = NPIECE[j]
                    hp = GL // npc
                    ab_r = [
                        t.rearrange("(ct p) s -> p ct s", p=128) for t in ctg_h[j]
                    ]
                    # earlier-gathered pieces first so PE starts sooner
                    order = sorted(range(CT), key=lambda t: (t % 8) // hp)
                    for st4 in range(4):
                        st = j * 4 + st4
                        ssl = slice(st4 * 128, (st4 + 1) * 128)
                        psums = []
                        for cc in range(2):
                            p = ops.tile([128, 512], FP32, name=f"op{cc}", tag=f"op{cc}")
                            psums.append(p)
                        cps = []
                        for a in range(npc):
                            cp = csbpool.tile(
                                [128, 2 * hp, 128], FP16, name=f"cp{a}", tag=f"c_sb{a}"
                            )
                            nc.sync.dma_start(out=cp, in_=ab_r[a][:, :, ssl])
                            cps.append(cp)

                        def csl(t):
                            piece = cps[(t % 8) // hp]
                            return piece[:, (t // 8) * hp + (t % hp), :]

                        for i, ct in enumerate(order):
                            for cc in range(2):
                                nc.tensor.matmul(
                                    out=psums[cc],
                                    lhsT=csl(ct),
                                    rhs=wo_t[:, ct, cc * 512:(cc + 1) * 512],
                                    start=(i == 0),
                                    stop=(i == CT - 1),
                                )
                        o_sb = osbpool.tile([128, OC], FP32, name="o_sb", tag="o_sb")
                        nc.scalar.copy(out=o_sb[:, 0:512], in_=psums[0])
                        nc.vector.tensor_copy(out=o_sb[:, 512:1024], in_=psums[1])
                        nc.sync.dma_start(out=out[st * 128:(st + 1) * 128, :], in_=o_sb)

                emit_attn(0)
                emit_attn(1)
                emit_o(0)
                emit_attn(2)
                emit_o(1)
                emit_attn(3)
                emit_o(2)
                emit_o(3)

    nc.compile()
    return nc


_NC_CACHE = {}


def _get_program():
    if "nc" not in _NC_CACHE:
        _NC_CACHE["nc"] = _build_program()
    return _NC_CACHE["nc"]


def kernel(query, key, value, mask, position_ids, Wq, Wk, Wv, Wo, **kw):
    query = np.asarray(query, dtype=np.float32)
    key = np.asarray(key, dtype=np.float32)
    value = np.asarray(value, dtype=np.float32)
    mask = np.asarray(mask)
    position_ids = np.asarray(position_ids)
    Wq = np.asarray(Wq, dtype=np.float32)
    Wk = np.asarray(Wk, dtype=np.float32)
    Wv = np.asarray(Wv, dtype=np.float32)
    Wo = np.asarray(Wo, dtype=np.float32)

    # rope tables from actual position_ids (applied to query only)
    pos = position_ids.astype(np.float64)  # (S,)
    freq = np.arange(0, RD, 2, dtype=np.float64)
    inv_freq = 1.0 / (10000.0 ** (freq / RD))  # (RD/2,)
    pe = pos[:, None] * inv_freq[None, :]      # (S, RD/2=256)
    cosT_half = np.ascontiguousarray(np.cos(pe).T.astype(np.float16))  # (256, S)
    sinT_half = np.ascontiguousarray(np.sin(pe).T.astype(np.float16))
    cosT_id = np.ones((RD // 2, S), np.float16)
    sinT_id = np.zeros((RD // 2, S), np.float16)

    ones_sq = np.ones((128, 128), np.float16)
    WoT = Wo.T.astype(np.float16)

    in_maps = []
    for core in range(8):
        b, h = core // 2, core % 2
        hs = slice(h * CL, (h + 1) * CL)
        biasm = np.where(mask[b] == 0, np.float32(MASK_BIAS), np.float32(0.0))
        in_maps.append({
            "xq": np.ascontiguousarray(query[b].T),
            "xk": np.ascontiguousarray(key[b].T),
            "xv": np.ascontiguousarray(value[b].T),
            "wq": np.ascontiguousarray(Wq[hs, :].T),
            "wk": np.ascontiguousarray(Wk[hs, :].T),
            "wv": np.ascontiguousarray(Wv[hs, :].T),
            "wo": np.ascontiguousarray(WoT[:, hs]),
            "cosT": cosT_half if h == 0 else cosT_id,
            "sinT": sinT_half if h == 0 else sinT_id,
            "biasm": np.ascontiguousarray(biasm.reshape(NT, 128).T),
            "ones_in": ones_sq,
        })

    nc = _get_program()
    res = run_bass_kernel_spmd(nc, in_maps, core_ids=list(range(8)))
    _NC_CACHE["last_res"] = res

    out = np.empty((B, S, D), np.float32)
    for core in range(8):
        b, h = core // 2, core % 2
        out[b][:, h * OC:(h + 1) * OC] = res.results[core]["out"]
    return out
